# revision 10
# baseline (speedup 1.0000x reference)
"""Trainium2 Bass kernel for nn_EqStftPBC (STFT perturbation-based compensation).

Per (batch b, mode m):
  X = STFT(x); C_n2 = X*conj(roll(X,n2)); U_n2 = circ(w[:,n2]) @ C (+ time-roll);
  V_n2 = U_n2 * roll(X,n2); delta_f = sum_n2 V_n2; y = ISTFT(delta)*P
8 cores = (b x m x n2-half); per-core data-only variation (S/M stacks).

v3: t-major work layout (addr = t*CHJ + j) so every big elementwise op and the
time-roll are flat; all big elementwise on vector (co-running gpsimd halves
DVE throughput); critical input DMAs isolated on the sync queue; j-sum in PSUM
via zero-stride matmul dst; overlap-add via selector matmuls.
"""

import numpy as np
from ml_dtypes import bfloat16

import concourse.bass as bass
import concourse.bacc as bacc
import concourse.mybir as mybir
import concourse.tile as tile

F = 80
T = 51
TP = 52
HOP = 40
L = 2080
NJ = 20
NCH = 2
CHJ = NJ // NCH   # 10
PBK = 5           # j's per R/U psum bank
BK = T * CHJ      # 510, t-major component block (addr = t*CHJ + j)
FP32 = mybir.dt.float32
BF16 = mybir.dt.bfloat16

N2_LISTS = [list(range(19, -1, -1)), list(range(-1, -21, -1))]


def _dft_consts():
    j = np.arange(F)
    W = np.exp(-2j * np.pi * np.outer(j, j) / F)
    G = np.exp(+2j * np.pi * np.outer(j, j) / F) / F
    return W, G


def build_program(debug=False):
    nc = bacc.Bacc("TRN2", target_bir_lowering=False, debug=debug)

    # xf = [fiN | fr | fi] frames, pre-framed on host (pure reshape)
    xf = nc.dram_tensor("xf", [F, 3 * T], BF16, kind="ExternalInput")
    fr_c = nc.dram_tensor("fr_c", [F, 2 * F], BF16, kind="ExternalInput")
    # gr_c = [Gr | Gi | P1 | P2]  (P1/P2 = overlap-add selector matrices)
    gr_c = nc.dram_tensor("gr_c", [F, 2 * F + 2 * HOP], BF16, kind="ExternalInput")
    smat = nc.dram_tensor("smat", [F, NJ * F], BF16, kind="ExternalInput")
    mst = nc.dram_tensor("mst", [F, NJ * 2 * F], BF16, kind="ExternalInput")
    svec = nc.dram_tensor("svec", [HOP, 52], FP32, kind="ExternalInput")
    yv = nc.dram_tensor("yv", [HOP, 2 * 52], FP32, kind="ExternalOutput")

    MUL = mybir.AluOpType.mult
    ADD = mybir.AluOpType.add
    SUB = mybir.AluOpType.subtract
    CPY = mybir.ActivationFunctionType.Copy

    with tile.TileContext(nc) as tc:
        with (
            tc.tile_pool(name="const", bufs=1) as cpool,
            tc.tile_pool(name="work", bufs=1) as wpool,
            tc.tile_pool(name="ps_x", bufs=1, space="PSUM") as ps_x,
            tc.tile_pool(name="ps_r", bufs=2, space="PSUM") as ps_r,
            tc.tile_pool(name="ps_u", bufs=2, space="PSUM") as ps_u,
            tc.tile_pool(name="ps_d", bufs=1, space="PSUM") as ps_d,
            tc.tile_pool(name="ps_y", bufs=1, space="PSUM") as ps_y,
        ):
            # ---- input DMAs; sync queue carries ONLY the critical path
            # (queue stripes interleave across requests, so a big tensor on
            #  the same queue delays small critical ones)
            frm = wpool.tile([F, 3 * T], BF16, tag="frm")
            nc.sync.dma_start(frm[:, :], xf[:, :])
            Fc = cpool.tile([F, 2 * F], BF16, tag="Fc")
            nc.sync.dma_start(Fc[:, :], fr_c[:, :])
            Ssb = cpool.tile([F, NJ * F], BF16, tag="Ssb")
            for q in range(4):   # R stationaries, in consumption order
                nc.sync.dma_start(Ssb[:, q * PBK * F:(q + 1) * PBK * F],
                                  smat[:, q * PBK * F:(q + 1) * PBK * F])
            Msb = cpool.tile([F, NJ * 2 * F], BF16, tag="Msb")
            nc.gpsimd.dma_start(Msb[:, 0:CHJ * 2 * F], mst[:, 0:CHJ * 2 * F])
            nc.scalar.dma_start(Msb[:, CHJ * 2 * F:], mst[:, CHJ * 2 * F:])
            Gc = cpool.tile([F, 2 * F + 2 * HOP], BF16, tag="Gc")
            nc.gpsimd.dma_start(Gc[:, :], gr_c[:, :])
            sv = cpool.tile([HOP, 52], FP32, tag="sv")
            nc.scalar.dma_start(sv[:, :], svec[:, :])

            # D eviction buffer with zero guard columns: per c2 block of 53:
            # [z | t0..t50 | z];  memset once, eviction fills the middle.
            Dsb = wpool.tile([F, 2 * 53], BF16, tag="Dsb")
            nc.gpsimd.memset(Dsb[:, :], 0.0)

            # ---- STFT (fp32 accum) -> X bf16 [Xr(52) | Xi(52)] ----
            Xp = ps_x.tile([F, 2 * T], FP32, tag="Xp")
            nc.tensor.matmul(Xp[:, :], Fc[:, 0:F], frm[:, T:3 * T], start=True, stop=False)
            nc.tensor.matmul(Xp[:, :], Fc[:, F:2 * F], frm[:, 0:2 * T], start=False, stop=True)
            Xsb = wpool.tile([F, 2 * TP], BF16, tag="Xsb")
            Xsv = Xsb[:, :].rearrange("p (c t) -> p c t", c=2)
            nc.scalar.activation(Xsv[:, :, 0:T],
                                 Xp[:, :].rearrange("p (c t) -> p c t", c=2), CPY)
            Xrhs = bass.AP(tensor=Xsb[:, :].tensor, offset=Xsb[:, :].offset,
                           ap=[[2 * TP, F], [TP, 2], [1, T]])
            # X tiled over j, t-major: Xt*[f, t*CHJ + j] = X[f, t]
            Xtr = wpool.tile([F, BK], BF16, tag="Xtr")
            Xti = wpool.tile([F, BK], BF16, tag="Xti")
            nc.scalar.activation(
                Xtr[:, :].rearrange("p (t j) -> p t j", t=T),
                Xsb[:, 0:T, None].to_broadcast([F, T, CHJ]), CPY)
            nc.scalar.activation(
                Xti[:, :].rearrange("p (t j) -> p t j", t=T),
                Xsb[:, TP:TP + T, None].to_broadcast([F, T, CHJ]), CPY)

            # ---- per-chunk tiles (component blocks of BK, t-major) ----
            Rsb, Csb, Usb, Vsb = [], [], [], []
            for c in range(NCH):
                Rsb.append(wpool.tile([F, 2 * BK], BF16, tag=f"Rsb{c}", name=f"Rsb{c}"))
                Csb.append(wpool.tile([F, 3 * BK], BF16, tag=f"Csb{c}", name=f"Csb{c}"))
                Usb.append(wpool.tile([F, 2 * BK], BF16, tag=f"Usb{c}", name=f"Usb{c}"))
                Vsb.append(wpool.tile([F, 3 * BK], BF16, tag=f"Vsb{c}", name=f"Vsb{c}"))
            Ue = [wpool.tile([F, 2 * BK], BF16, tag=f"Ue{c}", name=f"Ue{c}")
                  for c in range(NCH)]
            sA = wpool.tile([F, BK], BF16, tag="sA")
            sB = wpool.tile([F, BK], BF16, tag="sB")
            sC = wpool.tile([F, BK], BF16, tag="sC")
            sD = wpool.tile([F, BK], BF16, tag="sD")
            sA2 = wpool.tile([F, BK], BF16, tag="sA2")
            sB2 = wpool.tile([F, BK], BF16, tag="sB2")
            sC2 = wpool.tile([F, BK], BF16, tag="sC2")
            sD2 = wpool.tile([F, BK], BF16, tag="sD2")

            TT = nc.vector.tensor_tensor

            def evict_ap(dst_tile, bk):
                # psum [s(PBK), c2(2), t(T)] -> t-major dst addr c2*BK + t*CHJ + (bk*PBK+s)
                return bass.AP(tensor=dst_tile[:, :].tensor,
                               offset=dst_tile[:, :].offset + bk * PBK,
                               ap=[[2 * BK, F], [1, PBK], [BK, 2], [CHJ, T]])

            def r_stage(c):
                """R_j = roll(X, n2_j): permutation matmuls, scalar evict."""
                for bk in range(CHJ // PBK):
                    Rp = ps_r.tile([F, PBK * 2 * T], FP32, tag="Rp")
                    for s in range(PBK):
                        j = c * CHJ + bk * PBK + s
                        nc.tensor.matmul(Rp[:, s * 2 * T:(s + 1) * 2 * T],
                                         Ssb[:, j * F:(j + 1) * F],
                                         Xrhs, start=True, stop=True)
                    nc.scalar.activation(
                        evict_ap(Rsb[c], bk),
                        Rp[:, :].rearrange("p (s c2 t) -> p s c2 t", s=PBK, c2=2),
                        CPY)

            def c_stage(c):
                """C_pre = X * conj(R) -> Csb blocks [CiN | Cr | Ci]."""
                Rc, Cc = Rsb[c], Csb[c]
                Rrf = Rc[:, 0:BK]
                Rif = Rc[:, BK:2 * BK]
                a, b_, c_, d_ = (sA, sB, sC, sD) if c == 0 else (sA2, sB2, sC2, sD2)
                TT(a[:, :], Xtr[:, :], Rrf, MUL)
                TT(b_[:, :], Xti[:, :], Rif, MUL)
                TT(Cc[:, BK:2 * BK], a[:, :], b_[:, :], ADD)
                TT(c_[:, :], Xti[:, :], Rrf, MUL)
                TT(d_[:, :], Xtr[:, :], Rif, MUL)
                TT(Cc[:, 2 * BK:3 * BK], c_[:, :], d_[:, :], SUB)
                nc.scalar.activation(Cc[:, 0:BK], Cc[:, 2 * BK:3 * BK], CPY, scale=-1.0)

            def u_mm(c):
                """Up_j = Mr@[Cr|Ci] + Mi@[CiN|Cr]; scalar evict to Ue."""
                Cc = Csb[c]
                for bk in range(CHJ // PBK):
                    Up = ps_u.tile([F, PBK * 2 * T], FP32, tag="Up")
                    for s in range(PBK):
                        jj = bk * PBK + s
                        j = c * CHJ + jj
                        rhs1 = bass.AP(tensor=Cc[:, :].tensor,
                                       offset=Cc[:, :].offset + BK + jj,
                                       ap=[[3 * BK, F], [BK, 2], [CHJ, T]])
                        rhs2 = bass.AP(tensor=Cc[:, :].tensor,
                                       offset=Cc[:, :].offset + jj,
                                       ap=[[3 * BK, F], [BK, 2], [CHJ, T]])
                        nc.tensor.matmul(Up[:, s * 2 * T:(s + 1) * 2 * T],
                                         Msb[:, (2 * j) * F:(2 * j + 1) * F],
                                         rhs1, start=True, stop=False)
                        nc.tensor.matmul(Up[:, s * 2 * T:(s + 1) * 2 * T],
                                         Msb[:, (2 * j + 1) * F:(2 * j + 2) * F],
                                         rhs2, start=False, stop=True)
                    nc.scalar.activation(
                        evict_ap(Ue[c], bk),
                        Up[:, :].rearrange("p (s c2 t) -> p s c2 t", s=PBK, c2=2),
                        CPY)

            def u_roll(c):
                """Flat time-roll per component block: U[t] = Ue[t] + Ue[t-1]."""
                Uc = Usb[c]
                for blk in range(2):
                    o = blk * BK
                    TT(Uc[:, o + CHJ:o + BK],
                       Ue[c][:, o + CHJ:o + BK], Ue[c][:, o:o + BK - CHJ], ADD)
                    TT(Uc[:, o:o + CHJ],
                       Ue[c][:, o:o + CHJ], Ue[c][:, o + BK - CHJ:o + BK], ADD)

            def v_stage(c):
                """V = U * R -> Vsb blocks [ViN | Vr | Vi]."""
                Rc, Uc, Vc = Rsb[c], Usb[c], Vsb[c]
                Rrf, Rif = Rc[:, 0:BK], Rc[:, BK:2 * BK]
                Urf, Uif = Uc[:, 0:BK], Uc[:, BK:2 * BK]
                a, b_, c_, d_ = (sA, sB, sC, sD) if c == 0 else (sA2, sB2, sC2, sD2)
                TT(a[:, :], Urf, Rrf, MUL)
                TT(b_[:, :], Uif, Rif, MUL)
                TT(Vc[:, BK:2 * BK], a[:, :], b_[:, :], SUB)
                TT(c_[:, :], Urf, Rif, MUL)
                TT(d_[:, :], Uif, Rrf, MUL)
                TT(Vc[:, 2 * BK:3 * BK], c_[:, :], d_[:, :], ADD)
                nc.scalar.activation(Vc[:, 0:BK], Vc[:, 2 * BK:3 * BK], CPY, scale=-1.0)

            Dp = ps_d.tile([F, 2 * T], FP32, tag="Dp")

            def g_stage(c, start, stop):
                """D += sum_j G @ V_j : zero-stride dst accumulates j in PSUM."""
                Vc = Vsb[c]
                dst = bass.AP(tensor=Dp[:, :].tensor, offset=Dp[:, :].offset,
                              ap=[[2 * T, F], [0, PBK], [T, 2], [1, T]])
                for gpass in range(2):
                    for h in range(CHJ // PBK):
                        base = (BK if gpass == 0 else 0) + h * PBK
                        rhs = bass.AP(tensor=Vc[:, :].tensor,
                                      offset=Vc[:, :].offset + base,
                                      ap=[[3 * BK, F], [1, PBK], [BK, 2], [CHJ, T]])
                        nc.tensor.matmul(
                            dst, Gc[:, gpass * F:(gpass + 1) * F], rhs,
                            start=(start and gpass == 0 and h == 0),
                            stop=(stop and gpass == 1 and h == CHJ // PBK - 1))

            # ---------- pipelined issue order ----------
            # vector queue sees: C0, C1, roll0, V0, roll1, V1 (no stalls on
            # U matmuls blocking later C work)
            r_stage(0)
            r_stage(1)
            c_stage(0)
            u_mm(0)
            c_stage(1)
            u_roll(0)
            v_stage(0)
            u_mm(1)
            g_stage(0, start=True, stop=False)
            u_roll(1)
            v_stage(1)
            g_stage(1, start=False, stop=True)

            # ---------- tail: evict D, overlap-add via selector matmuls ----------
            dce = bass.AP(tensor=Dsb[:, :].tensor, offset=Dsb[:, :].offset + 1,
                          ap=[[2 * 53, F], [53, 2], [1, T]])
            nc.scalar.activation(dce, Dp[:, :].rearrange("p (c t) -> p c t", c=2), CPY)
            Yp = ps_y.tile([HOP, 2 * 52], FP32, tag="Yp")
            # y[tau, c2, tp] = D[tau, c2, tp] + D[tau+40, c2, tp-1]
            dstY = bass.AP(tensor=Yp[:, :].tensor, offset=Yp[:, :].offset,
                           ap=[[2 * 52, HOP], [52, 2], [1, 52]])
            rhs1 = bass.AP(tensor=Dsb[:, :].tensor, offset=Dsb[:, :].offset + 1,
                           ap=[[2 * 53, F], [53, 2], [1, 52]])
            rhs2 = bass.AP(tensor=Dsb[:, :].tensor, offset=Dsb[:, :].offset,
                           ap=[[2 * 53, F], [53, 2], [1, 52]])
            nc.tensor.matmul(dstY, Gc[:, 2 * F:2 * F + HOP], rhs1,
                             start=True, stop=False)
            nc.tensor.matmul(dstY, Gc[:, 2 * F + HOP:2 * F + 2 * HOP], rhs2,
                             start=False, stop=True)
            Y = wpool.tile([HOP, 2 * 52], FP32, tag="Y")
            TT(Y[:, :].rearrange("p (c t) -> p c t", c=2),
               Yp[:, :].rearrange("p (c t) -> p c t", c=2),
               sv[:, None, :].to_broadcast([HOP, 2, 52]), MUL)
            nc.sync.dma_start(yv[:, :], Y[:, :])
    return nc


# ---------------- host side ----------------

def _host_consts():
    W, G = _dft_consts()
    fr_c = np.concatenate([W.real, W.imag], axis=1).astype(bfloat16)
    P1 = np.zeros((F, HOP), np.float32)
    P2 = np.zeros((F, HOP), np.float32)
    P1[np.arange(HOP), np.arange(HOP)] = 1.0
    P2[HOP + np.arange(HOP), np.arange(HOP)] = 1.0
    gr_c = np.concatenate([G.real, G.imag, P1, P2], axis=1).astype(bfloat16)
    cov = np.zeros(L)
    idx = (np.arange(T)[:, None] * HOP + np.arange(F)[None, :]).reshape(-1)
    np.add.at(cov, idx, 1.0)
    cov = np.where(cov > 0, cov, 1.0)
    return fr_c, gr_c, cov


def _smat_for(n2_list):
    S = np.zeros((NJ, F, F), np.float32)
    g = np.arange(F)
    for j, n2 in enumerate(n2_list):
        S[j, (g - n2) % F, g] = 1.0
    return np.ascontiguousarray(S.transpose(1, 0, 2).reshape(F, NJ * F)).astype(bfloat16)


def _mst_for(n2_list, w2):
    Ms = np.zeros((NJ, 2, F, F), np.float32)
    g = np.arange(F)[:, None]
    f = np.arange(F)[None, :]
    n1 = ((f - g + 20) % F) - 20
    valid = (n1 >= -20) & (n1 <= 19)
    n1c = np.clip(n1 + 20, 0, 39)
    for j, n2 in enumerate(n2_list):
        col = w2[:, n2 + 20]
        Ms[j, 0] = np.where(valid, col.real[n1c], 0.0)
        Ms[j, 1] = np.where(valid, col.imag[n1c], 0.0)
    return np.ascontiguousarray(
        Ms.transpose(2, 0, 1, 3).reshape(F, NJ * 2 * F)).astype(bfloat16)


def _frame(sig):
    idx = np.arange(T)[None, :] * HOP + np.arange(F)[:, None]   # [j, t]
    return sig[idx].astype(np.float32)


def make_in_maps(x_real, x_imag, task_info, w_real, w_imag):
    fr_c, gr_c, cov = _host_consts()
    b, _, m = x_real.shape
    P = np.power(10.0, task_info[:, 0] / 10.0) / m
    w2 = (np.asarray(w_real) + 1j * np.asarray(w_imag)).reshape(40, 40)
    smats = [_smat_for(nl) for nl in N2_LISTS]
    msts = [_mst_for(nl, w2) for nl in N2_LISTS]

    tp = np.arange(52)[None, :]
    tau = np.arange(HOP)[:, None]
    l = HOP * tp + tau
    svs = [(P[bb] / cov[l]).astype(np.float32) for bb in range(b)]

    in_maps, shards = [], []
    for bb in range(b):
        for mm in range(m):
            fr_ = _frame(x_real[bb, :, mm])
            fi_ = _frame(x_imag[bb, :, mm])
            xfv = np.concatenate([-fi_, fr_, fi_], axis=1).astype(bfloat16)
            for h in range(2):
                in_maps.append({
                    "xf": xfv,
                    "fr_c": fr_c,
                    "gr_c": gr_c,
                    "smat": smats[h],
                    "mst": msts[h],
                    "svec": svs[bb],
                })
                shards.append((bb, mm, h))
    return in_maps, shards, P, cov


_NC_CACHE = {}


def kernel(x_real, x_imag, task_info, w_real, w_imag, b_real, b_imag):
    x_real = np.asarray(x_real)
    x_imag = np.asarray(x_imag)
    task_info = np.asarray(task_info)
    b, Lx, m = x_real.shape
    assert (b, Lx, m) == (2, L, 2)

    if "nc" not in _NC_CACHE:
        nc_ = build_program(debug=False)
        nc_.compile()
        _NC_CACHE["nc"] = nc_
    nc = _NC_CACHE["nc"]

    in_maps, shards, P, cov = make_in_maps(x_real, x_imag, task_info, w_real, w_imag)
    from concourse.bass_utils import run_bass_kernel_spmd
    res = run_bass_kernel_spmd(nc, in_maps, list(range(8))).results

    x = (x_real + 1j * x_imag).astype(np.complex64)
    out = x.copy()
    bias = complex(np.asarray(b_real)[0], np.asarray(b_imag)[0])
    bias_sig = np.zeros(L, np.complex64)
    bias_sig[np.arange(T) * HOP] = bias
    bias_sig /= cov
    for i, (bb, mm, h) in enumerate(shards):
        yvv = res[i]["yv"]          # [40, 104] = [tau, (yr(52) | yi(52))]
        yr = yvv[:, 0:52].T.ravel()[:L]
        yi = yvv[:, 52:104].T.ravel()[:L]
        out[bb, :, mm] += yr + 1j * yi
    for bb in range(b):
        for mm in range(m):
            out[bb, :, mm] += (P[bb] * bias_sig).astype(np.complex64)
    return out[:, 20:L - 20, :]


# revision 11
# speedup vs baseline: 1.4462x; 1.4462x over previous
"""Trainium2 Bass kernel for nn_EqStftPBC (STFT perturbation-based compensation).

Per (batch b, mode m):
  X = STFT(x); C_n2 = X*conj(roll(X,n2)); U_n2 = circ(w[:,n2]) @ C (+ time-roll);
  V_n2 = U_n2 * roll(X,n2); delta_f = sum_n2 V_n2; y = ISTFT(delta)*P
8 cores = (b x m x n2-half); per-core data-only variation (S/M stacks).

v4: j-major layout (contiguous evicts + fast matmul rhs); ALL big elementwise
ops on vector only (a co-running gpsimd halves DVE throughput); time-roll as
one flat TT per chunk via ghost slots; j-sum in PSUM via zero-stride matmul
dst; overlap-add via selector matmuls; critical DMAs isolated on sync queue.
"""

import numpy as np
from ml_dtypes import bfloat16

import concourse.bass as bass
import concourse.bacc as bacc
import concourse.mybir as mybir
import concourse.tile as tile

F = 80
T = 51
TP = 52          # per-j slot stride (51 data + 1 pad/ghost)
HOP = 40
L = 2080
NJ = 20
NCH = 2
CHJ = NJ // NCH  # 10
PBK = 5          # j's per R/U psum bank
BL = CHJ * TP    # 520
FP32 = mybir.dt.float32
BF16 = mybir.dt.bfloat16

N2_LISTS = [list(range(19, -1, -1)), list(range(-1, -21, -1))]


def _dft_consts():
    j = np.arange(F)
    W = np.exp(-2j * np.pi * np.outer(j, j) / F)
    G = np.exp(+2j * np.pi * np.outer(j, j) / F) / F
    return W, G


def build_program(debug=False):
    nc = bacc.Bacc("TRN2", target_bir_lowering=False, debug=debug)

    # xf = [fiN | fr | fi] frames, pre-framed on host (pure reshape)
    xf = nc.dram_tensor("xf", [F, 3 * T], BF16, kind="ExternalInput")
    fr_c = nc.dram_tensor("fr_c", [F, 2 * F], BF16, kind="ExternalInput")
    # gr_c = [Gr | Gi | P1 | P2]  (P1/P2 = overlap-add selector matrices)
    gr_c = nc.dram_tensor("gr_c", [F, 2 * F + 2 * HOP], BF16, kind="ExternalInput")
    smat = nc.dram_tensor("smat", [F, NJ * F], BF16, kind="ExternalInput")
    mst = nc.dram_tensor("mst", [F, NJ * 2 * F], BF16, kind="ExternalInput")
    svec = nc.dram_tensor("svec", [HOP, 52], FP32, kind="ExternalInput")
    yv = nc.dram_tensor("yv", [HOP, 2 * 52], FP32, kind="ExternalOutput")

    MUL = mybir.AluOpType.mult
    ADD = mybir.AluOpType.add
    SUB = mybir.AluOpType.subtract
    CPY = mybir.ActivationFunctionType.Copy

    with tile.TileContext(nc) as tc:
        with (
            tc.tile_pool(name="const", bufs=1) as cpool,
            tc.tile_pool(name="work", bufs=1) as wpool,
            tc.tile_pool(name="ps_x", bufs=1, space="PSUM") as ps_x,
            tc.tile_pool(name="ps_r", bufs=2, space="PSUM") as ps_r,
            tc.tile_pool(name="ps_u", bufs=2, space="PSUM") as ps_u,
            tc.tile_pool(name="ps_d", bufs=1, space="PSUM") as ps_d,
            tc.tile_pool(name="ps_y", bufs=1, space="PSUM") as ps_y,
        ):
            # ---- input DMAs; sync queue carries only the critical path
            frm = wpool.tile([F, 3 * T], BF16, tag="frm")
            nc.sync.dma_start(frm[:, :], xf[:, :])
            Fc = cpool.tile([F, 2 * F], BF16, tag="Fc")
            nc.sync.dma_start(Fc[:, :], fr_c[:, :])
            Ssb = cpool.tile([F, NJ * F], BF16, tag="Ssb")
            for q in range(4):   # R stationaries, in consumption order
                nc.sync.dma_start(Ssb[:, q * PBK * F:(q + 1) * PBK * F],
                                  smat[:, q * PBK * F:(q + 1) * PBK * F])
            Msb = cpool.tile([F, NJ * 2 * F], BF16, tag="Msb")
            nc.gpsimd.dma_start(Msb[:, 0:CHJ * 2 * F], mst[:, 0:CHJ * 2 * F])
            nc.scalar.dma_start(Msb[:, CHJ * 2 * F:], mst[:, CHJ * 2 * F:])
            Gc = cpool.tile([F, 2 * F + 2 * HOP], BF16, tag="Gc")
            nc.gpsimd.dma_start(Gc[:, :], gr_c[:, :])
            sv = cpool.tile([HOP, 52], FP32, tag="sv")
            nc.scalar.dma_start(sv[:, :], svec[:, :])

            # D eviction buffer with zero guard columns: per c2 block of 53:
            # [z | t0..t50 | z];  memset once, eviction fills the middle.
            Dsb = wpool.tile([F, 2 * 53], BF16, tag="Dsb")
            nc.gpsimd.memset(Dsb[:, :], 0.0)

            # ---- STFT (fp32 accum) -> X bf16 [Xr(52) | Xi(52)] ----
            Xp = ps_x.tile([F, 2 * T], FP32, tag="Xp")
            nc.tensor.matmul(Xp[:, :], Fc[:, 0:F], frm[:, T:3 * T], start=True, stop=False)
            nc.tensor.matmul(Xp[:, :], Fc[:, F:2 * F], frm[:, 0:2 * T], start=False, stop=True)
            Xsb = wpool.tile([F, 2 * TP], BF16, tag="Xsb")
            Xsv = Xsb[:, :].rearrange("p (c t) -> p c t", c=2)
            nc.scalar.activation(Xsv[:, :, 0:T],
                                 Xp[:, :].rearrange("p (c t) -> p c t", c=2), CPY)
            Xrhs = bass.AP(tensor=Xsb[:, :].tensor, offset=Xsb[:, :].offset,
                           ap=[[2 * TP, F], [TP, 2], [1, T]])
            # X tiled over j (flat TT operands for the C stage)
            Xtr = wpool.tile([F, BL], BF16, tag="Xtr")
            Xti = wpool.tile([F, BL], BF16, tag="Xti")
            nc.scalar.activation(
                Xtr[:, :].rearrange("p (j t) -> p j t", j=CHJ),
                Xsb[:, None, 0:TP].to_broadcast([F, CHJ, TP]), CPY)
            nc.scalar.activation(
                Xti[:, :].rearrange("p (j t) -> p j t", j=CHJ),
                Xsb[:, None, TP:2 * TP].to_broadcast([F, CHJ, TP]), CPY)

            # ---- per-chunk tiles ----
            Rsb, Csb, Usb, Vsb = [], [], [], []
            for c in range(NCH):
                Rsb.append(wpool.tile([F, 2 * BL], BF16, tag=f"Rsb{c}", name=f"Rsb{c}"))
                Csb.append(wpool.tile([F, 3 * BL], BF16, tag=f"Csb{c}", name=f"Csb{c}"))
                Usb.append(wpool.tile([F, 2 * BL], BF16, tag=f"Usb{c}", name=f"Usb{c}"))
                Vsb.append(wpool.tile([F, 3 * BL], BF16, tag=f"Vsb{c}", name=f"Vsb{c}"))
            # Ue has one leading ghost slot: data at 1 + c2*BL + j*TP + t
            Ue = [wpool.tile([F, 1 + 2 * BL], BF16, tag=f"Ue{c}", name=f"Ue{c}")
                  for c in range(NCH)]
            sA = wpool.tile([F, BL], BF16, tag="sA")
            sB = wpool.tile([F, BL], BF16, tag="sB")
            sC = wpool.tile([F, BL], BF16, tag="sC")
            sD = wpool.tile([F, BL], BF16, tag="sD")
            sA2 = wpool.tile([F, BL], BF16, tag="sA2")
            sB2 = wpool.tile([F, BL], BF16, tag="sB2")
            sC2 = wpool.tile([F, BL], BF16, tag="sC2")
            sD2 = wpool.tile([F, BL], BF16, tag="sD2")

            TT = nc.vector.tensor_tensor

            def r_stage(c):
                """R_j = roll(X, n2_j): permutation matmuls, scalar evict."""
                Rc = Rsb[c]
                for bk in range(CHJ // PBK):
                    Rp = ps_r.tile([F, PBK * 2 * T], FP32, tag="Rp")
                    for s in range(PBK):
                        j = c * CHJ + bk * PBK + s
                        nc.tensor.matmul(Rp[:, s * 2 * T:(s + 1) * 2 * T],
                                         Ssb[:, j * F:(j + 1) * F],
                                         Xrhs, start=True, stop=True)
                    dst = bass.AP(tensor=Rc[:, :].tensor,
                                  offset=Rc[:, :].offset + bk * PBK * TP,
                                  ap=[[2 * BL, F], [TP, PBK], [BL, 2], [1, T]])
                    nc.scalar.activation(
                        dst, Rp[:, :].rearrange("p (s c2 t) -> p s c2 t", s=PBK, c2=2),
                        CPY)

            def c_stage(c):
                """C_pre = X * conj(R) -> Csb blocks [CiN | Cr | Ci]."""
                Rc, Cc = Rsb[c], Csb[c]
                Rrf = Rc[:, 0:BL]
                Rif = Rc[:, BL:2 * BL]
                a, b_, c_, d_ = (sA, sB, sC, sD) if c == 0 else (sA2, sB2, sC2, sD2)
                TT(a[:, :], Xtr[:, :], Rrf, MUL)
                TT(b_[:, :], Xti[:, :], Rif, MUL)
                TT(Cc[:, BL:2 * BL], a[:, :], b_[:, :], ADD)
                TT(c_[:, :], Xti[:, :], Rrf, MUL)
                TT(d_[:, :], Xtr[:, :], Rif, MUL)
                TT(Cc[:, 2 * BL:3 * BL], c_[:, :], d_[:, :], SUB)
                nc.scalar.activation(Cc[:, 0:BL], Cc[:, 2 * BL:3 * BL], CPY, scale=-1.0)

            def u_mm(c):
                """Up_j = Mr@[Cr|Ci] + Mi@[CiN|Cr]; scalar evict; ghost fill."""
                Cc = Csb[c]
                for bk in range(CHJ // PBK):
                    Up = ps_u.tile([F, PBK * 2 * T], FP32, tag="Up")
                    for s in range(PBK):
                        jj = bk * PBK + s
                        j = c * CHJ + jj
                        rhs1 = bass.AP(tensor=Cc[:, :].tensor,
                                       offset=Cc[:, :].offset + BL + jj * TP,
                                       ap=[[3 * BL, F], [BL, 2], [1, T]])
                        rhs2 = bass.AP(tensor=Cc[:, :].tensor,
                                       offset=Cc[:, :].offset + jj * TP,
                                       ap=[[3 * BL, F], [BL, 2], [1, T]])
                        nc.tensor.matmul(Up[:, s * 2 * T:(s + 1) * 2 * T],
                                         Msb[:, (2 * j) * F:(2 * j + 1) * F],
                                         rhs1, start=True, stop=False)
                        nc.tensor.matmul(Up[:, s * 2 * T:(s + 1) * 2 * T],
                                         Msb[:, (2 * j + 1) * F:(2 * j + 2) * F],
                                         rhs2, start=False, stop=True)
                    dst = bass.AP(tensor=Ue[c][:, :].tensor,
                                  offset=Ue[c][:, :].offset + 1 + bk * PBK * TP,
                                  ap=[[1 + 2 * BL, F], [TP, PBK], [BL, 2], [1, T]])
                    nc.scalar.activation(
                        dst, Up[:, :].rearrange("p (s c2 t) -> p s c2 t", s=PBK, c2=2),
                        CPY)
                # ghost slots: pos (1 + c2*BL + j*TP) - 1  <-  value at +51
                gdst = bass.AP(tensor=Ue[c][:, :].tensor, offset=Ue[c][:, :].offset,
                               ap=[[1 + 2 * BL, F], [BL, 2], [TP, CHJ], [1, 1]])
                gsrc = bass.AP(tensor=Ue[c][:, :].tensor,
                               offset=Ue[c][:, :].offset + T,
                               ap=[[1 + 2 * BL, F], [BL, 2], [TP, CHJ], [1, 1]])
                nc.scalar.activation(gdst, gsrc, CPY)

            def u_roll(c):
                """One flat TT: U = Ue[1:] + Ue[:-1] (ghosts make wrap correct)."""
                TT(Usb[c][:, 0:2 * BL],
                   Ue[c][:, 1:1 + 2 * BL], Ue[c][:, 0:2 * BL], ADD)

            def v_stage(c):
                """V = U * R -> Vsb blocks [ViN | Vr | Vi]."""
                Rc, Uc, Vc = Rsb[c], Usb[c], Vsb[c]
                Rrf, Rif = Rc[:, 0:BL], Rc[:, BL:2 * BL]
                Urf, Uif = Uc[:, 0:BL], Uc[:, BL:2 * BL]
                a, b_, c_, d_ = (sA, sB, sC, sD) if c == 0 else (sA2, sB2, sC2, sD2)
                TT(a[:, :], Urf, Rrf, MUL)
                TT(b_[:, :], Uif, Rif, MUL)
                TT(Vc[:, BL:2 * BL], a[:, :], b_[:, :], SUB)
                TT(c_[:, :], Urf, Rif, MUL)
                TT(d_[:, :], Uif, Rrf, MUL)
                TT(Vc[:, 2 * BL:3 * BL], c_[:, :], d_[:, :], ADD)
                nc.scalar.activation(Vc[:, 0:BL], Vc[:, 2 * BL:3 * BL], CPY, scale=-1.0)

            Dp = ps_d.tile([F, 2 * T], FP32, tag="Dp")

            def g_stage(c, start, stop):
                """D += sum_j G @ V_j : zero-stride dst accumulates j in PSUM."""
                Vc = Vsb[c]
                dst = bass.AP(tensor=Dp[:, :].tensor, offset=Dp[:, :].offset,
                              ap=[[2 * T, F], [0, PBK], [T, 2], [1, T]])
                for gpass in range(2):
                    for h in range(CHJ // PBK):
                        base = (BL if gpass == 0 else 0) + h * PBK * TP
                        rhs = bass.AP(tensor=Vc[:, :].tensor,
                                      offset=Vc[:, :].offset + base,
                                      ap=[[3 * BL, F], [TP, PBK], [BL, 2], [1, T]])
                        nc.tensor.matmul(
                            dst, Gc[:, gpass * F:(gpass + 1) * F], rhs,
                            start=(start and gpass == 0 and h == 0),
                            stop=(stop and gpass == 1 and h == CHJ // PBK - 1))

            # ---------- pipelined issue order ----------
            # vector queue: C0(6), C1(6), roll0, V0(6), roll1, V1(6), Y
            r_stage(0)
            r_stage(1)
            c_stage(0)
            u_mm(0)
            c_stage(1)
            u_roll(0)
            v_stage(0)
            u_mm(1)
            g_stage(0, start=True, stop=False)
            u_roll(1)
            v_stage(1)
            g_stage(1, start=False, stop=True)

            # ---------- tail: evict D, overlap-add via selector matmuls ----------
            dce = bass.AP(tensor=Dsb[:, :].tensor, offset=Dsb[:, :].offset + 1,
                          ap=[[2 * 53, F], [53, 2], [1, T]])
            nc.scalar.activation(dce, Dp[:, :].rearrange("p (c t) -> p c t", c=2), CPY)
            Yp = ps_y.tile([HOP, 2 * 52], FP32, tag="Yp")
            # y[tau, c2, tp] = D[tau, c2, tp] + D[tau+40, c2, tp-1]
            dstY = bass.AP(tensor=Yp[:, :].tensor, offset=Yp[:, :].offset,
                           ap=[[2 * 52, HOP], [52, 2], [1, 52]])
            rhs1 = bass.AP(tensor=Dsb[:, :].tensor, offset=Dsb[:, :].offset + 1,
                           ap=[[2 * 53, F], [53, 2], [1, 52]])
            rhs2 = bass.AP(tensor=Dsb[:, :].tensor, offset=Dsb[:, :].offset,
                           ap=[[2 * 53, F], [53, 2], [1, 52]])
            nc.tensor.matmul(dstY, Gc[:, 2 * F:2 * F + HOP], rhs1,
                             start=True, stop=False)
            nc.tensor.matmul(dstY, Gc[:, 2 * F + HOP:2 * F + 2 * HOP], rhs2,
                             start=False, stop=True)
            Y = wpool.tile([HOP, 2 * 52], FP32, tag="Y")
            TT(Y[:, :].rearrange("p (c t) -> p c t", c=2),
               Yp[:, :].rearrange("p (c t) -> p c t", c=2),
               sv[:, None, :].to_broadcast([HOP, 2, 52]), MUL)
            nc.sync.dma_start(yv[:, :], Y[:, :])
    return nc


# ---------------- host side ----------------

def _host_consts():
    W, G = _dft_consts()
    fr_c = np.concatenate([W.real, W.imag], axis=1).astype(bfloat16)
    P1 = np.zeros((F, HOP), np.float32)
    P2 = np.zeros((F, HOP), np.float32)
    P1[np.arange(HOP), np.arange(HOP)] = 1.0
    P2[HOP + np.arange(HOP), np.arange(HOP)] = 1.0
    gr_c = np.concatenate([G.real, G.imag, P1, P2], axis=1).astype(bfloat16)
    cov = np.zeros(L)
    idx = (np.arange(T)[:, None] * HOP + np.arange(F)[None, :]).reshape(-1)
    np.add.at(cov, idx, 1.0)
    cov = np.where(cov > 0, cov, 1.0)
    return fr_c, gr_c, cov


def _smat_for(n2_list):
    S = np.zeros((NJ, F, F), np.float32)
    g = np.arange(F)
    for j, n2 in enumerate(n2_list):
        S[j, (g - n2) % F, g] = 1.0
    return np.ascontiguousarray(S.transpose(1, 0, 2).reshape(F, NJ * F)).astype(bfloat16)


def _mst_for(n2_list, w2):
    Ms = np.zeros((NJ, 2, F, F), np.float32)
    g = np.arange(F)[:, None]
    f = np.arange(F)[None, :]
    n1 = ((f - g + 20) % F) - 20
    valid = (n1 >= -20) & (n1 <= 19)
    n1c = np.clip(n1 + 20, 0, 39)
    for j, n2 in enumerate(n2_list):
        col = w2[:, n2 + 20]
        Ms[j, 0] = np.where(valid, col.real[n1c], 0.0)
        Ms[j, 1] = np.where(valid, col.imag[n1c], 0.0)
    return np.ascontiguousarray(
        Ms.transpose(2, 0, 1, 3).reshape(F, NJ * 2 * F)).astype(bfloat16)


def _frame(sig):
    idx = np.arange(T)[None, :] * HOP + np.arange(F)[:, None]   # [j, t]
    return sig[idx].astype(np.float32)


def make_in_maps(x_real, x_imag, task_info, w_real, w_imag):
    fr_c, gr_c, cov = _host_consts()
    b, _, m = x_real.shape
    P = np.power(10.0, task_info[:, 0] / 10.0) / m
    w2 = (np.asarray(w_real) + 1j * np.asarray(w_imag)).reshape(40, 40)
    smats = [_smat_for(nl) for nl in N2_LISTS]
    msts = [_mst_for(nl, w2) for nl in N2_LISTS]

    tp = np.arange(52)[None, :]
    tau = np.arange(HOP)[:, None]
    l = HOP * tp + tau
    svs = [(P[bb] / cov[l]).astype(np.float32) for bb in range(b)]

    in_maps, shards = [], []
    for bb in range(b):
        for mm in range(m):
            fr_ = _frame(x_real[bb, :, mm])
            fi_ = _frame(x_imag[bb, :, mm])
            xfv = np.concatenate([-fi_, fr_, fi_], axis=1).astype(bfloat16)
            for h in range(2):
                in_maps.append({
                    "xf": xfv,
                    "fr_c": fr_c,
                    "gr_c": gr_c,
                    "smat": smats[h],
                    "mst": msts[h],
                    "svec": svs[bb],
                })
                shards.append((bb, mm, h))
    return in_maps, shards, P, cov


_NC_CACHE = {}


def kernel(x_real, x_imag, task_info, w_real, w_imag, b_real, b_imag):
    x_real = np.asarray(x_real)
    x_imag = np.asarray(x_imag)
    task_info = np.asarray(task_info)
    b, Lx, m = x_real.shape
    assert (b, Lx, m) == (2, L, 2)

    if "nc" not in _NC_CACHE:
        nc_ = build_program(debug=False)
        nc_.compile()
        _NC_CACHE["nc"] = nc_
    nc = _NC_CACHE["nc"]

    in_maps, shards, P, cov = make_in_maps(x_real, x_imag, task_info, w_real, w_imag)
    from concourse.bass_utils import run_bass_kernel_spmd
    res = run_bass_kernel_spmd(nc, in_maps, list(range(8))).results

    x = (x_real + 1j * x_imag).astype(np.complex64)
    out = x.copy()
    bias = complex(np.asarray(b_real)[0], np.asarray(b_imag)[0])
    bias_sig = np.zeros(L, np.complex64)
    bias_sig[np.arange(T) * HOP] = bias
    bias_sig /= cov
    for i, (bb, mm, h) in enumerate(shards):
        yvv = res[i]["yv"]          # [40, 104] = [tau, (yr(52) | yi(52))]
        yr = yvv[:, 0:52].T.ravel()[:L]
        yi = yvv[:, 52:104].T.ravel()[:L]
        out[bb, :, mm] += yr + 1j * yi
    for bb in range(b):
        for mm in range(m):
            out[bb, :, mm] += (P[bb] * bias_sig).astype(np.complex64)
    return out[:, 20:L - 20, :]


# revision 20
# speedup vs baseline: 1.5299x; 1.0579x over previous
"""Trainium2 Bass kernel for nn_EqStftPBC (STFT perturbation-based compensation).

Per (batch b, mode m):
  X = STFT(x); C_n2 = X*conj(roll(X,n2)); U_n2 = circ(w[:,n2]) @ C (+ time-roll);
  V_n2 = U_n2 * roll(X,n2); delta_f = sum_n2 V_n2; y = ISTFT(delta)*P
8 cores = (b x m x n2-half); per-core data-only variation (S/M stacks).

v4: j-major layout (contiguous evicts + fast matmul rhs); ALL big elementwise
ops on vector only (a co-running gpsimd halves DVE throughput); time-roll as
one flat TT per chunk via ghost slots; j-sum in PSUM via zero-stride matmul
dst; overlap-add via selector matmuls; critical DMAs isolated on sync queue.
"""

import numpy as np
from ml_dtypes import bfloat16

import concourse.bass as bass
import concourse.bacc as bacc
import concourse.mybir as mybir
import concourse.tile as tile

F = 80
T = 51
TP = 52          # per-j slot stride (51 data + 1 pad/ghost)
HOP = 40
L = 2080
NJ = 20
NCH = 2
CHJ = NJ // NCH  # 10
PBK = 5          # j's per R/U psum bank
BL = CHJ * TP    # 520
FP32 = mybir.dt.float32
BF16 = mybir.dt.bfloat16

N2_LISTS = [list(range(19, -1, -1)), list(range(-1, -21, -1))]


def _dft_consts():
    j = np.arange(F)
    W = np.exp(-2j * np.pi * np.outer(j, j) / F)
    G = np.exp(+2j * np.pi * np.outer(j, j) / F) / F
    return W, G


def build_program(debug=False):
    nc = bacc.Bacc("TRN2", target_bir_lowering=False, debug=debug)

    # crit = [xf frames (3T) | fr_c (2F)]: one DMA gates the STFT
    crit = nc.dram_tensor("crit", [F, 3 * T + 2 * F], BF16, kind="ExternalInput")
    # gr_c = [Gr | Gi | P1 | P2]  (P1/P2 = overlap-add selector matrices)
    gr_c = nc.dram_tensor("gr_c", [F, 2 * F + 2 * HOP], BF16, kind="ExternalInput")
    smat = nc.dram_tensor("smat", [F, NJ * F], BF16, kind="ExternalInput")
    mst = nc.dram_tensor("mst", [F, NJ * 2 * F], BF16, kind="ExternalInput")
    svec = nc.dram_tensor("svec", [HOP, 52], FP32, kind="ExternalInput")
    yv = nc.dram_tensor("yv", [HOP, 2 * 52], FP32, kind="ExternalOutput")

    MUL = mybir.AluOpType.mult
    ADD = mybir.AluOpType.add
    SUB = mybir.AluOpType.subtract
    CPY = mybir.ActivationFunctionType.Copy

    with tile.TileContext(nc) as tc:
        with (
            tc.tile_pool(name="const", bufs=1) as cpool,
            tc.tile_pool(name="work", bufs=1) as wpool,
            tc.tile_pool(name="ps_x", bufs=1, space="PSUM") as ps_x,
            tc.tile_pool(name="ps_r", bufs=2, space="PSUM") as ps_r,
            tc.tile_pool(name="ps_u", bufs=2, space="PSUM") as ps_u,
            tc.tile_pool(name="ps_d", bufs=1, space="PSUM") as ps_d,
            tc.tile_pool(name="ps_y", bufs=1, space="PSUM") as ps_y,
        ):
            # ---- input DMAs; sync queue carries ONLY the critical tensor
            Crit = wpool.tile([F, 3 * T + 2 * F], BF16, tag="Crit")
            nc.sync.dma_start(Crit[:, :], crit[:, :])
            FCO = 3 * T   # Fc column offset within Crit
            Ssb = cpool.tile([F, NJ * F], BF16, tag="Ssb")
            for q in range(4):   # R stationaries, in consumption order
                nc.scalar.dma_start(Ssb[:, q * PBK * F:(q + 1) * PBK * F],
                                    smat[:, q * PBK * F:(q + 1) * PBK * F])
            Msb = cpool.tile([F, NJ * 2 * F], BF16, tag="Msb")
            nc.gpsimd.dma_start(Msb[:, 0:CHJ * 2 * F], mst[:, 0:CHJ * 2 * F])
            nc.scalar.dma_start(Msb[:, CHJ * 2 * F:], mst[:, CHJ * 2 * F:])
            Gc = cpool.tile([F, 2 * F + 2 * HOP], BF16, tag="Gc")
            nc.gpsimd.dma_start(Gc[:, :], gr_c[:, :])
            sv = cpool.tile([HOP, 52], FP32, tag="sv")
            nc.scalar.dma_start(sv[:, :], svec[:, :])

            # D eviction buffer with zero guard columns: per c2 block of 53:
            # [z | t0..t50 | z];  memset once, eviction fills the middle.
            Dsb = wpool.tile([F, 2 * 53], BF16, tag="Dsb")
            nc.gpsimd.memset(Dsb[:, :], 0.0)

            # ---- STFT (fp32 accum) -> X bf16 [Xr(52) | Xi(52)] ----
            Xp = ps_x.tile([F, 2 * T], FP32, tag="Xp")
            nc.tensor.matmul(Xp[:, :], Crit[:, FCO:FCO + F], Crit[:, T:3 * T],
                             start=True, stop=False)
            nc.tensor.matmul(Xp[:, :], Crit[:, FCO + F:FCO + 2 * F], Crit[:, 0:2 * T],
                             start=False, stop=True)
            Xsb = wpool.tile([F, 2 * TP], BF16, tag="Xsb")
            Xsv = Xsb[:, :].rearrange("p (c t) -> p c t", c=2)
            nc.scalar.activation(Xsv[:, :, 0:T],
                                 Xp[:, :].rearrange("p (c t) -> p c t", c=2), CPY)
            Xrhs = bass.AP(tensor=Xsb[:, :].tensor, offset=Xsb[:, :].offset,
                           ap=[[2 * TP, F], [TP, 2], [1, T]])
            # X tiled over j (flat TT operands for the C stage)
            Xtr = wpool.tile([F, BL], BF16, tag="Xtr")
            Xti = wpool.tile([F, BL], BF16, tag="Xti")
            nc.scalar.activation(
                Xtr[:, :].rearrange("p (j t) -> p j t", j=CHJ),
                Xsb[:, None, 0:TP].to_broadcast([F, CHJ, TP]), CPY)
            nc.scalar.activation(
                Xti[:, :].rearrange("p (j t) -> p j t", j=CHJ),
                Xsb[:, None, TP:2 * TP].to_broadcast([F, CHJ, TP]), CPY)

            # ---- per-chunk tiles ----
            Rsb, Csb, Usb, Vsb = [], [], [], []
            for c in range(NCH):
                Rsb.append(wpool.tile([F, 2 * BL], BF16, tag=f"Rsb{c}", name=f"Rsb{c}"))
                Csb.append(wpool.tile([F, 3 * BL], BF16, tag=f"Csb{c}", name=f"Csb{c}"))
                Usb.append(wpool.tile([F, 2 * BL], BF16, tag=f"Usb{c}", name=f"Usb{c}"))
                Vsb.append(wpool.tile([F, 3 * BL], BF16, tag=f"Vsb{c}", name=f"Vsb{c}"))
            # Ue has one leading ghost slot: data at 1 + c2*BL + j*TP + t
            Ue = [wpool.tile([F, 1 + 2 * BL], BF16, tag=f"Ue{c}", name=f"Ue{c}")
                  for c in range(NCH)]
            sA = wpool.tile([F, BL], BF16, tag="sA")
            sB = wpool.tile([F, BL], BF16, tag="sB")
            sC = wpool.tile([F, BL], BF16, tag="sC")
            sD = wpool.tile([F, BL], BF16, tag="sD")
            sA2 = wpool.tile([F, BL], BF16, tag="sA2")
            sB2 = wpool.tile([F, BL], BF16, tag="sB2")
            sC2 = wpool.tile([F, BL], BF16, tag="sC2")
            sD2 = wpool.tile([F, BL], BF16, tag="sD2")

            TT = nc.vector.tensor_tensor

            def r_stage(c):
                """R_j = roll(X, n2_j): permutation matmuls, scalar evict."""
                Rc = Rsb[c]
                for bk in range(CHJ // PBK):
                    Rp = ps_r.tile([F, PBK * 2 * T], FP32, tag="Rp")
                    for s in range(PBK):
                        j = c * CHJ + bk * PBK + s
                        nc.tensor.matmul(Rp[:, s * 2 * T:(s + 1) * 2 * T],
                                         Ssb[:, j * F:(j + 1) * F],
                                         Xrhs, start=True, stop=True)
                    dst = bass.AP(tensor=Rc[:, :].tensor,
                                  offset=Rc[:, :].offset + bk * PBK * TP,
                                  ap=[[2 * BL, F], [TP, PBK], [BL, 2], [1, T]])
                    nc.scalar.activation(
                        dst, Rp[:, :].rearrange("p (s c2 t) -> p s c2 t", s=PBK, c2=2),
                        CPY)

            def c_stage(c):
                """C_pre = X * conj(R) -> Csb blocks [CiN | Cr | Ci]."""
                Rc, Cc = Rsb[c], Csb[c]
                Rrf = Rc[:, 0:BL]
                Rif = Rc[:, BL:2 * BL]
                a, b_, c_, d_ = (sA, sB, sC, sD) if c == 0 else (sA2, sB2, sC2, sD2)
                # imag chain first so the scalar CiN negate overlaps the real
                # chain and the U matmuls start right after the last TT
                TT(c_[:, :], Xti[:, :], Rrf, MUL)
                TT(d_[:, :], Xtr[:, :], Rif, MUL)
                TT(Cc[:, 2 * BL:3 * BL], c_[:, :], d_[:, :], SUB)
                nc.scalar.activation(Cc[:, 0:BL], Cc[:, 2 * BL:3 * BL], CPY, scale=-1.0)
                TT(a[:, :], Xtr[:, :], Rrf, MUL)
                TT(b_[:, :], Xti[:, :], Rif, MUL)
                TT(Cc[:, BL:2 * BL], a[:, :], b_[:, :], ADD)

            def u_mm(c):
                """Up_j = Mr@[Cr|Ci] + Mi@[CiN|Cr]; scalar evict; ghost fill."""
                Cc = Csb[c]
                for bk in range(CHJ // PBK):
                    Up = ps_u.tile([F, PBK * 2 * T], FP32, tag="Up")
                    for s in range(PBK):
                        jj = bk * PBK + s
                        j = c * CHJ + jj
                        rhs1 = bass.AP(tensor=Cc[:, :].tensor,
                                       offset=Cc[:, :].offset + BL + jj * TP,
                                       ap=[[3 * BL, F], [BL, 2], [1, T]])
                        rhs2 = bass.AP(tensor=Cc[:, :].tensor,
                                       offset=Cc[:, :].offset + jj * TP,
                                       ap=[[3 * BL, F], [BL, 2], [1, T]])
                        nc.tensor.matmul(Up[:, s * 2 * T:(s + 1) * 2 * T],
                                         Msb[:, (2 * j) * F:(2 * j + 1) * F],
                                         rhs1, start=True, stop=False)
                        nc.tensor.matmul(Up[:, s * 2 * T:(s + 1) * 2 * T],
                                         Msb[:, (2 * j + 1) * F:(2 * j + 2) * F],
                                         rhs2, start=False, stop=True)
                    dst = bass.AP(tensor=Ue[c][:, :].tensor,
                                  offset=Ue[c][:, :].offset + 1 + bk * PBK * TP,
                                  ap=[[1 + 2 * BL, F], [TP, PBK], [BL, 2], [1, T]])
                    nc.scalar.activation(
                        dst, Up[:, :].rearrange("p (s c2 t) -> p s c2 t", s=PBK, c2=2),
                        CPY)
                # ghost slots: pos (1 + c2*BL + j*TP) - 1  <-  value at +51
                gdst = bass.AP(tensor=Ue[c][:, :].tensor, offset=Ue[c][:, :].offset,
                               ap=[[1 + 2 * BL, F], [BL, 2], [TP, CHJ], [1, 1]])
                gsrc = bass.AP(tensor=Ue[c][:, :].tensor,
                               offset=Ue[c][:, :].offset + T,
                               ap=[[1 + 2 * BL, F], [BL, 2], [TP, CHJ], [1, 1]])
                nc.scalar.activation(gdst, gsrc, CPY)

            def u_roll(c):
                """One flat TT: U = Ue[1:] + Ue[:-1] (ghosts make wrap correct)."""
                TT(Usb[c][:, 0:2 * BL],
                   Ue[c][:, 1:1 + 2 * BL], Ue[c][:, 0:2 * BL], ADD)

            def v_stage(c):
                """V = U * R -> Vsb blocks [ViN | Vr | Vi]."""
                Rc, Uc, Vc = Rsb[c], Usb[c], Vsb[c]
                Rrf, Rif = Rc[:, 0:BL], Rc[:, BL:2 * BL]
                Urf, Uif = Uc[:, 0:BL], Uc[:, BL:2 * BL]
                a, b_, c_, d_ = (sA, sB, sC, sD) if c == 0 else (sA2, sB2, sC2, sD2)
                # imag chain first: ViN (scalar) overlaps the Vr chain, so the
                # G matmuls wait less after the last TT
                TT(c_[:, :], Urf, Rif, MUL)
                TT(d_[:, :], Uif, Rrf, MUL)
                TT(Vc[:, 2 * BL:3 * BL], c_[:, :], d_[:, :], ADD)
                nc.scalar.activation(Vc[:, 0:BL], Vc[:, 2 * BL:3 * BL], CPY, scale=-1.0)
                TT(a[:, :], Urf, Rrf, MUL)
                TT(b_[:, :], Uif, Rif, MUL)
                TT(Vc[:, BL:2 * BL], a[:, :], b_[:, :], SUB)

            Dp = ps_d.tile([F, 2 * T], FP32, tag="Dp")

            def g_stage(c, start, stop):
                """D += sum_j G @ V_j : zero-stride dst accumulates j in PSUM."""
                Vc = Vsb[c]
                dst = bass.AP(tensor=Dp[:, :].tensor, offset=Dp[:, :].offset,
                              ap=[[2 * T, F], [0, PBK], [T, 2], [1, T]])
                for gpass in range(2):
                    for h in range(CHJ // PBK):
                        base = (BL if gpass == 0 else 0) + h * PBK * TP
                        rhs = bass.AP(tensor=Vc[:, :].tensor,
                                      offset=Vc[:, :].offset + base,
                                      ap=[[3 * BL, F], [TP, PBK], [BL, 2], [1, T]])
                        nc.tensor.matmul(
                            dst, Gc[:, gpass * F:(gpass + 1) * F], rhs,
                            start=(start and gpass == 0 and h == 0),
                            stop=(stop and gpass == 1 and h == CHJ // PBK - 1))

            # ---------- pipelined issue order ----------
            # vector queue: C0(6), C1(6), roll0, V0(6), roll1, V1(6), Y
            r_stage(0)
            r_stage(1)
            c_stage(0)
            u_mm(0)
            c_stage(1)
            u_roll(0)
            v_stage(0)
            u_mm(1)
            g_stage(0, start=True, stop=False)
            u_roll(1)
            v_stage(1)
            g_stage(1, start=False, stop=True)

            # ---------- tail: evict D, overlap-add via selector matmuls ----------
            dce = bass.AP(tensor=Dsb[:, :].tensor, offset=Dsb[:, :].offset + 1,
                          ap=[[2 * 53, F], [53, 2], [1, T]])
            nc.scalar.activation(dce, Dp[:, :].rearrange("p (c t) -> p c t", c=2), CPY)
            Yp = ps_y.tile([HOP, 2 * 52], FP32, tag="Yp")
            # y[tau, c2, tp] = D[tau, c2, tp] + D[tau+40, c2, tp-1]
            dstY = bass.AP(tensor=Yp[:, :].tensor, offset=Yp[:, :].offset,
                           ap=[[2 * 52, HOP], [52, 2], [1, 52]])
            rhs1 = bass.AP(tensor=Dsb[:, :].tensor, offset=Dsb[:, :].offset + 1,
                           ap=[[2 * 53, F], [53, 2], [1, 52]])
            rhs2 = bass.AP(tensor=Dsb[:, :].tensor, offset=Dsb[:, :].offset,
                           ap=[[2 * 53, F], [53, 2], [1, 52]])
            nc.tensor.matmul(dstY, Gc[:, 2 * F:2 * F + HOP], rhs1,
                             start=True, stop=False)
            nc.tensor.matmul(dstY, Gc[:, 2 * F + HOP:2 * F + 2 * HOP], rhs2,
                             start=False, stop=True)
            Y = wpool.tile([HOP, 2 * 52], FP32, tag="Y")
            TT(Y[:, :].rearrange("p (c t) -> p c t", c=2),
               Yp[:, :].rearrange("p (c t) -> p c t", c=2),
               sv[:, None, :].to_broadcast([HOP, 2, 52]), MUL)
            nc.sync.dma_start(yv[:, :], Y[:, :])
    return nc


# ---------------- host side ----------------

def _host_consts():
    W, G = _dft_consts()
    fr_c = np.concatenate([W.real, W.imag], axis=1).astype(bfloat16)
    P1 = np.zeros((F, HOP), np.float32)
    P2 = np.zeros((F, HOP), np.float32)
    P1[np.arange(HOP), np.arange(HOP)] = 1.0
    P2[HOP + np.arange(HOP), np.arange(HOP)] = 1.0
    gr_c = np.concatenate([G.real, G.imag, P1, P2], axis=1).astype(bfloat16)
    cov = np.zeros(L)
    idx = (np.arange(T)[:, None] * HOP + np.arange(F)[None, :]).reshape(-1)
    np.add.at(cov, idx, 1.0)
    cov = np.where(cov > 0, cov, 1.0)
    return fr_c, gr_c, cov


def _smat_for(n2_list):
    S = np.zeros((NJ, F, F), np.float32)
    g = np.arange(F)
    for j, n2 in enumerate(n2_list):
        S[j, (g - n2) % F, g] = 1.0
    return np.ascontiguousarray(S.transpose(1, 0, 2).reshape(F, NJ * F)).astype(bfloat16)


def _mst_for(n2_list, w2):
    Ms = np.zeros((NJ, 2, F, F), np.float32)
    g = np.arange(F)[:, None]
    f = np.arange(F)[None, :]
    n1 = ((f - g + 20) % F) - 20
    valid = (n1 >= -20) & (n1 <= 19)
    n1c = np.clip(n1 + 20, 0, 39)
    for j, n2 in enumerate(n2_list):
        col = w2[:, n2 + 20]
        Ms[j, 0] = np.where(valid, col.real[n1c], 0.0)
        Ms[j, 1] = np.where(valid, col.imag[n1c], 0.0)
    return np.ascontiguousarray(
        Ms.transpose(2, 0, 1, 3).reshape(F, NJ * 2 * F)).astype(bfloat16)


def _frame(sig):
    idx = np.arange(T)[None, :] * HOP + np.arange(F)[:, None]   # [j, t]
    return sig[idx].astype(np.float32)


def make_in_maps(x_real, x_imag, task_info, w_real, w_imag):
    fr_c, gr_c, cov = _host_consts()
    b, _, m = x_real.shape
    P = np.power(10.0, task_info[:, 0] / 10.0) / m
    w2 = (np.asarray(w_real) + 1j * np.asarray(w_imag)).reshape(40, 40)
    smats = [_smat_for(nl) for nl in N2_LISTS]
    msts = [_mst_for(nl, w2) for nl in N2_LISTS]

    tp = np.arange(52)[None, :]
    tau = np.arange(HOP)[:, None]
    l = HOP * tp + tau
    svs = [(P[bb] / cov[l]).astype(np.float32) for bb in range(b)]

    in_maps, shards = [], []
    for bb in range(b):
        for mm in range(m):
            fr_ = _frame(x_real[bb, :, mm])
            fi_ = _frame(x_imag[bb, :, mm])
            critv = np.concatenate(
                [np.concatenate([-fi_, fr_, fi_], axis=1).astype(bfloat16), fr_c],
                axis=1)
            for h in range(2):
                in_maps.append({
                    "crit": critv,
                    "gr_c": gr_c,
                    "smat": smats[h],
                    "mst": msts[h],
                    "svec": svs[bb],
                })
                shards.append((bb, mm, h))
    return in_maps, shards, P, cov


_NC_CACHE = {}


def kernel(x_real, x_imag, task_info, w_real, w_imag, b_real, b_imag):
    x_real = np.asarray(x_real)
    x_imag = np.asarray(x_imag)
    task_info = np.asarray(task_info)
    b, Lx, m = x_real.shape
    assert (b, Lx, m) == (2, L, 2)

    if "nc" not in _NC_CACHE:
        nc_ = build_program(debug=False)
        nc_.compile()
        _NC_CACHE["nc"] = nc_
    nc = _NC_CACHE["nc"]

    in_maps, shards, P, cov = make_in_maps(x_real, x_imag, task_info, w_real, w_imag)
    from concourse.bass_utils import run_bass_kernel_spmd
    res = run_bass_kernel_spmd(nc, in_maps, list(range(8))).results

    x = (x_real + 1j * x_imag).astype(np.complex64)
    out = x.copy()
    bias = complex(np.asarray(b_real)[0], np.asarray(b_imag)[0])
    bias_sig = np.zeros(L, np.complex64)
    bias_sig[np.arange(T) * HOP] = bias
    bias_sig /= cov
    for i, (bb, mm, h) in enumerate(shards):
        yvv = res[i]["yv"]          # [40, 104] = [tau, (yr(52) | yi(52))]
        yr = yvv[:, 0:52].T.ravel()[:L]
        yi = yvv[:, 52:104].T.ravel()[:L]
        out[bb, :, mm] += yr + 1j * yi
    for bb in range(b):
        for mm in range(m):
            out[bb, :, mm] += (P[bb] * bias_sig).astype(np.complex64)
    return out[:, 20:L - 20, :]


# revision 28
# speedup vs baseline: 1.6434x; 1.0742x over previous
"""Trainium2 Bass kernel for nn_EqStftPBC (STFT perturbation-based compensation).

Per (batch b, mode m):
  X = STFT(x); C_n2 = X*conj(roll(X,n2)); U_n2 = circ(w[:,n2]) @ C (+ time-roll);
  V_n2 = U_n2 * roll(X,n2); delta_f = sum_n2 V_n2; y = ISTFT(delta)*P
8 cores = (b x m x n2-half); per-core data-only variation (S/M stacks).

v4: j-major layout (contiguous evicts + fast matmul rhs); ALL big elementwise
ops on vector only (a co-running gpsimd halves DVE throughput); time-roll as
one flat TT per chunk via ghost slots; j-sum in PSUM via zero-stride matmul
dst; overlap-add via selector matmuls; critical DMAs isolated on sync queue.
"""

import numpy as np
from ml_dtypes import bfloat16

import concourse.bass as bass
import concourse.bacc as bacc
import concourse.mybir as mybir
import concourse.tile as tile

F = 80
T = 51
TP = 52          # per-j slot stride (51 data + 1 pad/ghost)
HOP = 40
L = 2080
NJ = 20
NCH = 2
CHJ = NJ // NCH  # 10
PBK = 5          # j's per R/U psum bank
BL = CHJ * TP    # 520
FP32 = mybir.dt.float32
BF16 = mybir.dt.bfloat16

N2_LISTS = [list(range(19, -1, -1)), list(range(-1, -21, -1))]


def _dft_consts():
    j = np.arange(F)
    W = np.exp(-2j * np.pi * np.outer(j, j) / F)
    G = np.exp(+2j * np.pi * np.outer(j, j) / F) / F
    return W, G


def build_program(debug=False):
    nc = bacc.Bacc("TRN2", target_bir_lowering=False, debug=debug)

    # crit = [xf frames (3T) | fr_c (2F)]: one DMA gates the STFT
    crit = nc.dram_tensor("crit", [F, 3 * T + 2 * F], BF16, kind="ExternalInput")
    # gr_c = [Gr | Gi | GiN | P1 | P2]  (GiN = -Gi; P1/P2 = overlap-add selectors)
    gr_c = nc.dram_tensor("gr_c", [F, 3 * F + 2 * HOP], BF16, kind="ExternalInput")
    smat = nc.dram_tensor("smat", [F, NJ * F], BF16, kind="ExternalInput")
    mst = nc.dram_tensor("mst", [F, NJ * 2 * F], BF16, kind="ExternalInput")
    svec = nc.dram_tensor("svec", [HOP, 52], FP32, kind="ExternalInput")
    yv = nc.dram_tensor("yv", [HOP, 2 * 52], FP32, kind="ExternalOutput")

    MUL = mybir.AluOpType.mult
    ADD = mybir.AluOpType.add
    SUB = mybir.AluOpType.subtract
    CPY = mybir.ActivationFunctionType.Copy

    with tile.TileContext(nc) as tc:
        with (
            tc.tile_pool(name="const", bufs=1) as cpool,
            tc.tile_pool(name="work", bufs=1) as wpool,
            tc.tile_pool(name="ps_x", bufs=1, space="PSUM") as ps_x,
            tc.tile_pool(name="ps_r", bufs=2, space="PSUM") as ps_r,
            tc.tile_pool(name="ps_u", bufs=2, space="PSUM") as ps_u,
            tc.tile_pool(name="ps_d", bufs=1, space="PSUM") as ps_d,
            tc.tile_pool(name="ps_y", bufs=1, space="PSUM") as ps_y,
        ):
            # ---- input DMAs; sync queue carries ONLY the critical tensor
            Crit = wpool.tile([F, 3 * T + 2 * F], BF16, tag="Crit")
            nc.sync.dma_start(Crit[:, :], crit[:, :])
            FCO = 3 * T   # Fc column offset within Crit
            Ssb = cpool.tile([F, NJ * F], BF16, tag="Ssb")
            for q in range(2):   # R stationaries, per-chunk, consumption order
                nc.scalar.dma_start(Ssb[:, q * CHJ * F:(q + 1) * CHJ * F],
                                    smat[:, q * CHJ * F:(q + 1) * CHJ * F])
            Msb = cpool.tile([F, NJ * 2 * F], BF16, tag="Msb")
            nc.gpsimd.dma_start(Msb[:, 0:CHJ * 2 * F], mst[:, 0:CHJ * 2 * F])
            nc.scalar.dma_start(Msb[:, CHJ * 2 * F:], mst[:, CHJ * 2 * F:])
            Gc = cpool.tile([F, 3 * F + 2 * HOP], BF16, tag="Gc")
            nc.gpsimd.dma_start(Gc[:, :], gr_c[:, :])
            sv = cpool.tile([HOP, 52], FP32, tag="sv")
            nc.gpsimd.dma_start(sv[:, :], svec[:, :])

            # D eviction buffer with zero guard columns: per c2 block of 53:
            # [z | t0..t50 | z];  memset once, eviction fills the middle.
            Dsb = wpool.tile([F, 2 * 53], BF16, tag="Dsb")
            nc.gpsimd.memset(Dsb[:, :], 0.0)

            # ---- STFT (fp32 accum) -> X bf16 [Xr(52) | Xi(52)] ----
            Xp = ps_x.tile([F, 2 * T], FP32, tag="Xp")
            nc.tensor.matmul(Xp[:, :], Crit[:, FCO:FCO + F], Crit[:, T:3 * T],
                             start=True, stop=False)
            nc.tensor.matmul(Xp[:, :], Crit[:, FCO + F:FCO + 2 * F], Crit[:, 0:2 * T],
                             start=False, stop=True)
            Xsb = wpool.tile([F, 2 * TP], BF16, tag="Xsb")
            Xsv = Xsb[:, :].rearrange("p (c t) -> p c t", c=2)
            nc.scalar.activation(Xsv[:, :, 0:T],
                                 Xp[:, :].rearrange("p (c t) -> p c t", c=2), CPY)
            Xrhs = bass.AP(tensor=Xsb[:, :].tensor, offset=Xsb[:, :].offset,
                           ap=[[2 * TP, F], [TP, 2], [1, T]])
            # X tiled over j (flat TT operands for the C stage), one ACTIVATE
            XtB = wpool.tile([F, 2 * BL], BF16, tag="XtB")
            xsrc = bass.AP(tensor=Xsb[:, :].tensor, offset=Xsb[:, :].offset,
                           ap=[[2 * TP, F], [TP, 2], [0, CHJ], [1, TP]])
            nc.scalar.activation(
                XtB[:, :].rearrange("p (c j t) -> p c j t", c=2, j=CHJ),
                xsrc, CPY)
            Xtr = XtB[:, 0:BL]
            Xti = XtB[:, BL:2 * BL]

            # ---- per-chunk tiles ----
            Rsb, Csb, Usb, Vsb = [], [], [], []
            for c in range(NCH):
                Rsb.append(wpool.tile([F, 2 * BL], BF16, tag=f"Rsb{c}", name=f"Rsb{c}"))
                Csb.append(wpool.tile([F, 3 * BL], BF16, tag=f"Csb{c}", name=f"Csb{c}"))
                Usb.append(wpool.tile([F, 2 * BL], BF16, tag=f"Usb{c}", name=f"Usb{c}"))
                Vsb.append(wpool.tile([F, 2 * BL], BF16, tag=f"Vsb{c}", name=f"Vsb{c}"))
            # Ue has one leading ghost slot: data at 1 + c2*BL + j*TP + t
            Ue = [wpool.tile([F, 1 + 2 * BL], BF16, tag=f"Ue{c}", name=f"Ue{c}")
                  for c in range(NCH)]
            sA = wpool.tile([F, BL], BF16, tag="sA")
            sB = wpool.tile([F, BL], BF16, tag="sB")
            sC = wpool.tile([F, BL], BF16, tag="sC")
            sD = wpool.tile([F, BL], BF16, tag="sD")
            sA2 = wpool.tile([F, BL], BF16, tag="sA2")
            sB2 = wpool.tile([F, BL], BF16, tag="sB2")
            sC2 = wpool.tile([F, BL], BF16, tag="sC2")
            sD2 = wpool.tile([F, BL], BF16, tag="sD2")

            TT = nc.vector.tensor_tensor

            def r_stage(c):
                """R_j = roll(X, n2_j): permutation matmuls, scalar evict."""
                Rc = Rsb[c]
                for bk in range(CHJ // PBK):
                    Rp = ps_r.tile([F, PBK * 2 * T], FP32, tag="Rp")
                    for s in range(PBK):
                        j = c * CHJ + bk * PBK + s
                        nc.tensor.matmul(Rp[:, s * 2 * T:(s + 1) * 2 * T],
                                         Ssb[:, j * F:(j + 1) * F],
                                         Xrhs, start=True, stop=True)
                    dst = bass.AP(tensor=Rc[:, :].tensor,
                                  offset=Rc[:, :].offset + bk * PBK * TP,
                                  ap=[[2 * BL, F], [TP, PBK], [BL, 2], [1, T]])
                    nc.scalar.activation(
                        dst, Rp[:, :].rearrange("p (s c2 t) -> p s c2 t", s=PBK, c2=2),
                        CPY)

            def c_stage(c):
                """C_pre = X * conj(R) -> Csb blocks [CiN | Cr | Ci]."""
                Rc, Cc = Rsb[c], Csb[c]
                Rrf = Rc[:, 0:BL]
                Rif = Rc[:, BL:2 * BL]
                a, b_, c_, d_ = (sA, sB, sC, sD) if c == 0 else (sA2, sB2, sC2, sD2)
                # imag chain first so the scalar CiN negate overlaps the real
                # chain and the U matmuls start right after the last TT
                TT(c_[:, :], Xti[:, :], Rrf, MUL)
                TT(d_[:, :], Xtr[:, :], Rif, MUL)
                TT(Cc[:, 2 * BL:3 * BL], c_[:, :], d_[:, :], SUB)
                nc.scalar.activation(Cc[:, 0:BL], Cc[:, 2 * BL:3 * BL], CPY, scale=-1.0)
                TT(a[:, :], Xtr[:, :], Rrf, MUL)
                TT(b_[:, :], Xti[:, :], Rif, MUL)
                TT(Cc[:, BL:2 * BL], a[:, :], b_[:, :], ADD)

            def u_mm(c):
                """Up_j = Mr@[Cr|Ci] + Mi@[CiN|Cr]; scalar evict; ghost fill."""
                Cc = Csb[c]
                for bk in range(CHJ // PBK):
                    Up = ps_u.tile([F, PBK * 2 * T], FP32, tag="Up")
                    for s in range(PBK):
                        jj = bk * PBK + s
                        j = c * CHJ + jj
                        rhs1 = bass.AP(tensor=Cc[:, :].tensor,
                                       offset=Cc[:, :].offset + BL + jj * TP,
                                       ap=[[3 * BL, F], [BL, 2], [1, T]])
                        rhs2 = bass.AP(tensor=Cc[:, :].tensor,
                                       offset=Cc[:, :].offset + jj * TP,
                                       ap=[[3 * BL, F], [BL, 2], [1, T]])
                        nc.tensor.matmul(Up[:, s * 2 * T:(s + 1) * 2 * T],
                                         Msb[:, (2 * j) * F:(2 * j + 1) * F],
                                         rhs1, start=True, stop=False)
                        nc.tensor.matmul(Up[:, s * 2 * T:(s + 1) * 2 * T],
                                         Msb[:, (2 * j + 1) * F:(2 * j + 2) * F],
                                         rhs2, start=False, stop=True)
                    dst = bass.AP(tensor=Ue[c][:, :].tensor,
                                  offset=Ue[c][:, :].offset + 1 + bk * PBK * TP,
                                  ap=[[1 + 2 * BL, F], [TP, PBK], [BL, 2], [1, T]])
                    nc.scalar.activation(
                        dst, Up[:, :].rearrange("p (s c2 t) -> p s c2 t", s=PBK, c2=2),
                        CPY)
                # ghost slots: pos (1 + c2*BL + j*TP) - 1  <-  value at +51
                gdst = bass.AP(tensor=Ue[c][:, :].tensor, offset=Ue[c][:, :].offset,
                               ap=[[1 + 2 * BL, F], [BL, 2], [TP, CHJ], [1, 1]])
                gsrc = bass.AP(tensor=Ue[c][:, :].tensor,
                               offset=Ue[c][:, :].offset + T,
                               ap=[[1 + 2 * BL, F], [BL, 2], [TP, CHJ], [1, 1]])
                nc.scalar.activation(gdst, gsrc, CPY)

            def u_roll(c):
                """One flat TT: U = Ue[1:] + Ue[:-1] (ghosts make wrap correct)."""
                TT(Usb[c][:, 0:2 * BL],
                   Ue[c][:, 1:1 + 2 * BL], Ue[c][:, 0:2 * BL], ADD)

            def v_stage(c):
                """V = U * R -> Vsb blocks [Vr | Vi] (no ViN: -Gi stationary)."""
                Rc, Uc, Vc = Rsb[c], Usb[c], Vsb[c]
                Rrf, Rif = Rc[:, 0:BL], Rc[:, BL:2 * BL]
                Urf, Uif = Uc[:, 0:BL], Uc[:, BL:2 * BL]
                a, b_, c_, d_ = (sA, sB, sC, sD) if c == 0 else (sA2, sB2, sC2, sD2)
                TT(c_[:, :], Urf, Rif, MUL)
                TT(d_[:, :], Uif, Rrf, MUL)
                TT(Vc[:, BL:2 * BL], c_[:, :], d_[:, :], ADD)
                TT(a[:, :], Urf, Rrf, MUL)
                TT(b_[:, :], Uif, Rif, MUL)
                TT(Vc[:, 0:BL], a[:, :], b_[:, :], SUB)

            Dp = ps_d.tile([F, 2 * T], FP32, tag="Dp")

            def g_stage(c, start, stop):
                """D += sum_j G @ V_j : zero-stride dst accumulates j in PSUM.

                Dr += Gr@Vr - Gi@Vi ; Di += Gr@Vi + Gi@Vr.  gpass0 does Gr on
                the [Vr|Vi] pair; gpass1 uses GiN=-Gi on Vi (Dr) and Gi on Vr
                (Di), so no negated V copy is ever materialized.
                """
                Vc = Vsb[c]
                dst2 = bass.AP(tensor=Dp[:, :].tensor, offset=Dp[:, :].offset,
                               ap=[[2 * T, F], [0, PBK], [T, 2], [1, T]])
                for h in range(CHJ // PBK):
                    rhs = bass.AP(tensor=Vc[:, :].tensor,
                                  offset=Vc[:, :].offset + h * PBK * TP,
                                  ap=[[2 * BL, F], [TP, PBK], [BL, 2], [1, T]])
                    nc.tensor.matmul(
                        dst2, Gc[:, 0:F], rhs,
                        start=(start and h == 0), stop=False)
                dstR = bass.AP(tensor=Dp[:, :].tensor, offset=Dp[:, :].offset,
                               ap=[[2 * T, F], [0, PBK], [1, T]])
                dstI = bass.AP(tensor=Dp[:, :].tensor, offset=Dp[:, :].offset + T,
                               ap=[[2 * T, F], [0, PBK], [1, T]])
                for h in range(CHJ // PBK):
                    rhsI = bass.AP(tensor=Vc[:, :].tensor,
                                   offset=Vc[:, :].offset + BL + h * PBK * TP,
                                   ap=[[2 * BL, F], [TP, PBK], [1, T]])
                    rhsR = bass.AP(tensor=Vc[:, :].tensor,
                                   offset=Vc[:, :].offset + h * PBK * TP,
                                   ap=[[2 * BL, F], [TP, PBK], [1, T]])
                    last = stop and h == CHJ // PBK - 1
                    nc.tensor.matmul(dstR, Gc[:, 2 * F:3 * F], rhsI,
                                     start=False, stop=False)
                    nc.tensor.matmul(dstI, Gc[:, F:2 * F], rhsR,
                                     start=False, stop=last)

            # ---------- pipelined issue order ----------
            # vector queue: C0(6), C1(6), roll0, V0(6), roll1, V1(6), Y
            r_stage(0)
            r_stage(1)
            c_stage(0)
            u_mm(0)
            c_stage(1)
            u_roll(0)
            v_stage(0)
            u_mm(1)
            g_stage(0, start=True, stop=False)
            u_roll(1)
            v_stage(1)
            g_stage(1, start=False, stop=True)

            # ---------- tail: evict D, overlap-add via selector matmuls ----------
            dce = bass.AP(tensor=Dsb[:, :].tensor, offset=Dsb[:, :].offset + 1,
                          ap=[[2 * 53, F], [53, 2], [1, T]])
            nc.scalar.activation(dce, Dp[:, :].rearrange("p (c t) -> p c t", c=2), CPY)
            Yp = ps_y.tile([HOP, 2 * 52], FP32, tag="Yp")
            # y[tau, c2, tp] = D[tau, c2, tp] + D[tau+40, c2, tp-1]
            dstY = bass.AP(tensor=Yp[:, :].tensor, offset=Yp[:, :].offset,
                           ap=[[2 * 52, HOP], [52, 2], [1, 52]])
            rhs1 = bass.AP(tensor=Dsb[:, :].tensor, offset=Dsb[:, :].offset + 1,
                           ap=[[2 * 53, F], [53, 2], [1, 52]])
            rhs2 = bass.AP(tensor=Dsb[:, :].tensor, offset=Dsb[:, :].offset,
                           ap=[[2 * 53, F], [53, 2], [1, 52]])
            nc.tensor.matmul(dstY, Gc[:, 3 * F:3 * F + HOP], rhs1,
                             start=True, stop=False)
            nc.tensor.matmul(dstY, Gc[:, 3 * F + HOP:3 * F + 2 * HOP], rhs2,
                             start=False, stop=True)
            Y = wpool.tile([HOP, 2 * 52], FP32, tag="Y")
            TT(Y[:, :].rearrange("p (c t) -> p c t", c=2),
               Yp[:, :].rearrange("p (c t) -> p c t", c=2),
               sv[:, None, :].to_broadcast([HOP, 2, 52]), MUL)
            nc.sync.dma_start(yv[:, :], Y[:, :])
    return nc


# ---------------- host side ----------------

def _host_consts():
    W, G = _dft_consts()
    fr_c = np.concatenate([W.real, W.imag], axis=1).astype(bfloat16)
    P1 = np.zeros((F, HOP), np.float32)
    P2 = np.zeros((F, HOP), np.float32)
    P1[np.arange(HOP), np.arange(HOP)] = 1.0
    P2[HOP + np.arange(HOP), np.arange(HOP)] = 1.0
    gr_c = np.concatenate([G.real, G.imag, -G.imag, P1, P2], axis=1).astype(bfloat16)
    cov = np.zeros(L)
    idx = (np.arange(T)[:, None] * HOP + np.arange(F)[None, :]).reshape(-1)
    np.add.at(cov, idx, 1.0)
    cov = np.where(cov > 0, cov, 1.0)
    return fr_c, gr_c, cov


def _smat_for(n2_list):
    S = np.zeros((NJ, F, F), np.float32)
    g = np.arange(F)
    for j, n2 in enumerate(n2_list):
        S[j, (g - n2) % F, g] = 1.0
    return np.ascontiguousarray(S.transpose(1, 0, 2).reshape(F, NJ * F)).astype(bfloat16)


def _mst_for(n2_list, w2):
    Ms = np.zeros((NJ, 2, F, F), np.float32)
    g = np.arange(F)[:, None]
    f = np.arange(F)[None, :]
    n1 = ((f - g + 20) % F) - 20
    valid = (n1 >= -20) & (n1 <= 19)
    n1c = np.clip(n1 + 20, 0, 39)
    for j, n2 in enumerate(n2_list):
        col = w2[:, n2 + 20]
        Ms[j, 0] = np.where(valid, col.real[n1c], 0.0)
        Ms[j, 1] = np.where(valid, col.imag[n1c], 0.0)
    return np.ascontiguousarray(
        Ms.transpose(2, 0, 1, 3).reshape(F, NJ * 2 * F)).astype(bfloat16)


def _frame(sig):
    idx = np.arange(T)[None, :] * HOP + np.arange(F)[:, None]   # [j, t]
    return sig[idx].astype(np.float32)


def make_in_maps(x_real, x_imag, task_info, w_real, w_imag):
    fr_c, gr_c, cov = _host_consts()
    b, _, m = x_real.shape
    P = np.power(10.0, task_info[:, 0] / 10.0) / m
    w2 = (np.asarray(w_real) + 1j * np.asarray(w_imag)).reshape(40, 40)
    smats = [_smat_for(nl) for nl in N2_LISTS]
    msts = [_mst_for(nl, w2) for nl in N2_LISTS]

    tp = np.arange(52)[None, :]
    tau = np.arange(HOP)[:, None]
    l = HOP * tp + tau
    svs = [(P[bb] / cov[l]).astype(np.float32) for bb in range(b)]

    in_maps, shards = [], []
    for bb in range(b):
        for mm in range(m):
            fr_ = _frame(x_real[bb, :, mm])
            fi_ = _frame(x_imag[bb, :, mm])
            critv = np.concatenate(
                [np.concatenate([-fi_, fr_, fi_], axis=1).astype(bfloat16), fr_c],
                axis=1)
            for h in range(2):
                in_maps.append({
                    "crit": critv,
                    "gr_c": gr_c,
                    "smat": smats[h],
                    "mst": msts[h],
                    "svec": svs[bb],
                })
                shards.append((bb, mm, h))
    return in_maps, shards, P, cov


_NC_CACHE = {}


def kernel(x_real, x_imag, task_info, w_real, w_imag, b_real, b_imag):
    x_real = np.asarray(x_real)
    x_imag = np.asarray(x_imag)
    task_info = np.asarray(task_info)
    b, Lx, m = x_real.shape
    assert (b, Lx, m) == (2, L, 2)

    if "nc" not in _NC_CACHE:
        nc_ = build_program(debug=False)
        nc_.compile()
        _NC_CACHE["nc"] = nc_
    nc = _NC_CACHE["nc"]

    in_maps, shards, P, cov = make_in_maps(x_real, x_imag, task_info, w_real, w_imag)
    from concourse.bass_utils import run_bass_kernel_spmd
    res = run_bass_kernel_spmd(nc, in_maps, list(range(8))).results

    x = (x_real + 1j * x_imag).astype(np.complex64)
    out = x.copy()
    bias = complex(np.asarray(b_real)[0], np.asarray(b_imag)[0])
    bias_sig = np.zeros(L, np.complex64)
    bias_sig[np.arange(T) * HOP] = bias
    bias_sig /= cov
    for i, (bb, mm, h) in enumerate(shards):
        yvv = res[i]["yv"]          # [40, 104] = [tau, (yr(52) | yi(52))]
        yr = yvv[:, 0:52].T.ravel()[:L]
        yi = yvv[:, 52:104].T.ravel()[:L]
        out[bb, :, mm] += yr + 1j * yi
    for bb in range(b):
        for mm in range(m):
            out[bb, :, mm] += (P[bb] * bias_sig).astype(np.complex64)
    return out[:, 20:L - 20, :]
